# revision 13
# baseline (speedup 1.0000x reference)
"""ColBERT late-interaction scoring kernel for Trainium2 (Bass/Tile).

score_b = sum_q max_k (2*Q@D^T - ||q||^2 - ||d||^2)
        = 2 * sum_q max_k (qd[q,k] - 0.5*d_sq[k])  -  ||Q_b||_F^2

Sharding: data-parallel over batch B=128 across 8 NeuronCores (16 each).

v3 — built for this backend's measured cost model:
  - DMA cost ~ 3.3us per non-contiguous descriptor run; contiguous
    DRAM->DRAM cast DMAs and whole-batch [512,128] DRAM->SBUF xbar
    transposes are ~free.  (v1's strided loads + per-tile xbar SBUF
    transposes, and v2's PE transposes, were the 25-50ms bottleneck.)
  - PE cost is ~120us per InstLdweights (stationary load); InstMatmult
    itself is ~free.  So: minimize weight loads.  64 qd matmuls (the
    mathematical floor) + 1 shared -0.5 stationary per 2-batch PSUM
    group for the d_sq bias broadcast; redundant Ldweights stripped.
  - DVE ~90ns/elem (too slow for the rowmax); Act/Pool ops ~free.
    Rowmax runs as a Pool tensor_tensor max tree over 4-batch groups.

Per-core per-rep pipeline:
  1. 2 contiguous cast DMAs: f32 DRAM -> bf16 DRAM staging
  2. 32 whole-batch xbar transposes: staging[b] [512,128] -> QT/DT
     [128(d), 512(L)] bf16 in SBUF (split across SP/Act HWDGE queues)
  3. Act: SQ = DT^2; QS[:, b] += sum(QT^2) (accum_out)
  4. PE per 2-batch group (8 psum banks): Ld(-0.5) + 8 bias matmuls
     (start) broadcasting -0.5*d_sq, then 8x [Ld(QT tile) + qd matmul]
     (stop)
  5. Act evicts each batch's 4 banks -> bf16 SBUF group buffer
  6. Pool max tree per 4 batches: [128, 16, 512] -> MX[:, 16 cols]
  Endgame: DVE sum over tiles, 2*msum - QS, ones-matmul partition sum.
"""

import numpy as np

B, LQ, LD, D = 128, 512, 512, 128
N_CORES = 8
BPC = B // N_CORES  # batches per core
NT = LQ // 128  # q tiles per batch

_compiled = {}


def _strip_redundant_ldweights(nc):
    """Remove InstLdweights that reload the stationary already in the PE
    array (identical weights AP, no other PE weight-modifying instruction
    in between). Sync info of a removed load moves to the next PE
    instruction, which executes at the same point in engine order."""
    import concourse.mybir as mybir

    for f in nc.m.functions:
        for blk in f.blocks:
            il = blk.instructions
            prev_key = None
            i = 0
            while i < len(il):
                inst = il[i]
                nm = type(inst).__name__
                if nm == "InstLdweights":
                    key = str(inst.ins[0])
                    if key == prev_key:
                        si = inst.sync_info
                        if si and (si.on_wait or si.on_update):
                            # find next PE instruction to carry the sync
                            j = i + 1
                            while j < len(il) and il[j].engine != inst.engine:
                                j += 1
                            if j < len(il):
                                nsi = il[j].sync_info
                                waits = list(si.on_wait or [])
                                upds = list(si.on_update or [])
                                if nsi:
                                    waits += list(nsi.on_wait or [])
                                    upds += list(nsi.on_update or [])
                                il[j].sync_info = mybir.SyncInfo(
                                    on_wait=waits, on_update=upds
                                )
                            else:
                                i += 1
                                continue
                        del il[i]
                        continue
                    prev_key = key
                elif nm == "InstMatmult":
                    pass
                elif inst.engine == mybir.EngineType.PE:
                    # any other PE instruction may clobber engine state
                    prev_key = None
                i += 1


def _split_multi_waits(nc):
    """This container's walrus accepts only ONE sem-wait per instruction
    (setupSyncWait: 'Too many sync wait commands'). Tile's wait assignment
    emits multi-wait instructions, so split: every extra wait moves onto a
    dedicated NoOp inserted just before the instruction on the same engine.
    Engine program order makes this semantically identical."""
    import concourse.mybir as mybir

    for f in nc.m.functions:
        for blk in f.blocks:
            il = blk.instructions
            i = 0
            while i < len(il):
                inst = il[i]
                si = inst.sync_info
                waits = list(si.on_wait) if si and si.on_wait else []
                if len(waits) > 1:
                    for w in waits[:-1]:
                        nop = mybir.InstNoOp(
                            name=nc.get_next_instruction_name(), ins=[], outs=[]
                        )
                        nop.engine = inst.engine
                        nop.sync_info = mybir.SyncInfo(on_wait=[w], on_update=[])
                        il.insert(i, nop)
                        i += 1
                    inst.sync_info = mybir.SyncInfo(
                        on_wait=[waits[-1]], on_update=si.on_update
                    )
                i += 1


def _build(reps: int = 1):
    import os
    ABL = set(os.environ.get("KABL", "").split(","))
    import concourse.bass as bass
    import concourse.mybir as mybir
    import concourse.tile as tile
    from concourse.bass import ts

    nc = bass.Bass()
    f32 = mybir.dt.float32
    bf16 = mybir.dt.bfloat16

    qe = nc.dram_tensor("qe", [BPC, LQ, D], f32, kind="ExternalInput")
    de = nc.dram_tensor("de", [BPC, LD, D], f32, kind="ExternalInput")
    qstage = nc.dram_tensor("qstage", [BPC, LQ, D], bf16, kind="Internal")
    dstage = nc.dram_tensor("dstage", [BPC, LD, D], bf16, kind="Internal")
    out = nc.dram_tensor("out", [1, BPC], f32, kind="ExternalOutput")

    with tile.TileContext(nc) as tc:
        with (
            tc.tile_pool(name="consts", bufs=1) as cpool,
            tc.tile_pool(name="work", bufs=3) as wpool,
            tc.tile_pool(name="acc", bufs=1) as apool,
            tc.tile_pool(name="ps", bufs=1, space="PSUM") as pspool,
        ):
            neg_half = cpool.tile([128, 128], bf16)
            nc.gpsimd.memset(neg_half, -0.5)
            ones_col = cpool.tile([128, 1], f32)
            nc.gpsimd.memset(ones_col, 1.0)

            MX = apool.tile([128, BPC * NT], f32)  # per (q-part, b*4+t) max
            QS = apool.tile([128, BPC], f32)  # per (d-part, b) sum(QT^2)

            for rep in range(reps):
                # Phases 1: contiguous cast DMAs (descriptor-cheap)
                if "nocast" not in ABL:
                    nc.gpsimd.dma_start(qstage[:, :, :], qe[:, :, :])
                    nc.gpsimd.dma_start(dstage[:, :, :], de[:, :, :])

                # Phase 2: all 32 whole-batch xbar transposes upfront,
                # split across the two HWDGE queues (SP + Act), fully
                # buffered so they stream back-to-back and hide under PE
                QTs, DTs, SQs = [], [], []
                for b in range(BPC):
                    QT = wpool.tile([128, LQ], bf16, tag="QT", bufs=BPC)
                    DT = wpool.tile([128, LD], bf16, tag="DT", bufs=BPC)
                    if "noxbar" not in ABL:
                        nc.sync.dma_start_transpose(QT, qstage[b])
                        nc.scalar.dma_start_transpose(DT, dstage[b])
                    QTs.append(QT)
                    DTs.append(DT)
                # Phase 3: SQ = DT*DT on Pool (bias rhs); ||Q||^2 on Act
                for b in range(BPC):
                    SQ = wpool.tile([128, LD], bf16, tag="SQ", bufs=BPC)
                    if "nosq" not in ABL:
                        nc.gpsimd.tensor_tensor(
                            SQ, DTs[b], DTs[b], op=mybir.AluOpType.mult
                        )
                        qsj = wpool.tile([128, LQ], bf16, tag="qsj")
                        nc.scalar.activation(
                            qsj, QTs[b],
                            mybir.ActivationFunctionType.Square,
                            accum_out=QS[:, b : b + 1],
                        )
                    SQs.append(SQ)

                # Phase 4-5: per 2-batch PSUM group; DVE rowmax straight
                # from PSUM (DVE ~90ns/elem = 2.9ms/rep, hidden under PE)
                do_pe = "nope" not in ABL
                do_bias = do_pe and "nobias" not in ABL and "nosq" not in ABL
                do_dve = do_pe and "nodve" not in ABL
                for b2 in range(BPC // 2):
                    b0 = 2 * b2
                    if do_pe:
                        pst = pspool.tile([128, 2, NT, 512], f32, tag="pst")
                    # 8 bias matmuls share one -0.5 stationary (strip pass
                    # removes the 7 redundant reloads)
                    if do_bias:
                      for i in range(2):
                        for t in range(NT):
                            nc.tensor.matmul(
                                pst[:, i, t, :], lhsT=neg_half, rhs=SQs[b0 + i],
                                start=True, stop=False,
                            )
                    # 8 qd matmuls, each loading its QT tile stationary.
                    # Evict each batch's 4 banks as soon as they finish so
                    # the next group's bias matmuls wait only on the second
                    # (not both) evictions: Act evict -> bf16 SBUF (fast
                    # psum release), DVE rowmax on SBUF bf16 (~92ns/elem)
                    EV = None
                    if do_dve:
                        EV = wpool.tile([128, 2, NT, 512], bf16, tag="EV", bufs=3)
                    for i in range(2):
                        if do_pe:
                            for t in range(NT):
                                nc.tensor.matmul(
                                    pst[:, i, t, :],
                                    lhsT=QTs[b0 + i][:, ts(t, 128)],
                                    rhs=DTs[b0 + i],
                                    start=not do_bias, stop=True,
                                )
                        if do_dve:
                            nc.scalar.copy(EV[:, i], pst[:, i])
                            nc.vector.reduce_max(
                                MX[:, (b0 + i) * NT : (b0 + i + 1) * NT],
                                EV[:, i], axis=mybir.AxisListType.X,
                            )

                # Endgame: score_b = 2 * sum_{q,t} MX - ||Q_b||^2
                if not do_dve or "nosq" in ABL:
                    continue
                msum = apool.tile([128, BPC], f32)
                nc.vector.reduce_sum(
                    msum, MX.rearrange("p (b t) -> p b t", t=NT),
                    axis=mybir.AxisListType.X,
                )
                sc = apool.tile([128, BPC], f32)
                msum2 = apool.tile([128, BPC], f32)
                nc.vector.tensor_scalar_mul(msum2, msum, 2.0)
                nc.vector.tensor_tensor(sc, msum2, QS, op=mybir.AluOpType.subtract)
                ps_s = pspool.tile([1, BPC], f32, tag="pst")
                nc.tensor.matmul(ps_s, lhsT=ones_col, rhs=sc, start=True, stop=True)
                score = apool.tile([1, BPC], f32)
                nc.vector.tensor_copy(score, ps_s)
                nc.sync.dma_start(out[:, :], score)

    import os
    if not os.environ.get("KNO_STRIP"):
        _strip_redundant_ldweights(nc)
    _split_multi_waits(nc)
    return nc


def kernel(query_embedding: np.ndarray, document_embedding: np.ndarray) -> np.ndarray:
    from concourse.bass_utils import run_bass_kernel_spmd

    if "nc" not in _compiled:
        _compiled["nc"] = _build()
    nc = _compiled["nc"]

    qe = np.ascontiguousarray(query_embedding, dtype=np.float32)
    de = np.ascontiguousarray(document_embedding, dtype=np.float32)
    in_maps = [
        {"qe": qe[c * BPC : (c + 1) * BPC], "de": de[c * BPC : (c + 1) * BPC]}
        for c in range(N_CORES)
    ]
    res = run_bass_kernel_spmd(nc, in_maps, core_ids=list(range(N_CORES)))
    return np.concatenate(
        [res.results[c]["out"].reshape(BPC) for c in range(N_CORES)]
    ).astype(np.float32)
